# revision 23
# baseline (speedup 1.0000x reference)
"""Differential Attention (B=1, S=2048, D=2048, H=16, DH=64) on 8 TRN2 NeuronCores.

Sharding: tensor-parallel over heads — 2 heads per core. Wq/Wk/Wv column-split,
attention fully local per head, Wo row-split; partial outputs summed on host
(the unshard step), which replaces the all-reduce.

Per-core pipeline (all matmuls fp32r except the PV stage in bf16):
  A1: projections qT/kT (in [dh, s] layout) and v (in [s, c] layout, bf16)
      from x^T (streamed from DRAM) and resident weight slices.
  Attention per 512-column s-block:
      scoresT[t, s] = kT.T-slices @ qT (K=64, row-packed pairs for the two
      score matrices), exp on the scalar engine (PSUM -> SBUF bf16),
      PV: out[s, 129] += expT_tile.T @ [v | 1]  — the ones column yields the
      softmax denominators for free.
  GroupNorm algebra: softmax(s1) - lam*softmax(s2) followed by GroupNorm is
  invariant to per-token positive scaling, so instead of normalizing both
  attentions we compute z = O1 - (lam*d1/d2)*O2 and use eps' = eps*d1^2 in
  the GN rsqrt (Newton iterations on the vector engine — keeps the scalar
  engine's activation table pinned to Exp). gn_weight*(1-lambda_init) is
  folded into Wo rows host-side; gn_bias's contribution is a constant vector
  added on host after the reduction.

PSUM (8 banks): A = [128,2,512]x2 (proj qk pairs, score pairs)
                B = [128,512]x2   (proj v accs j0/j1, PV accumulator pairs)
                C = [128,512]x2   (proj v accs j2/j3, transposes, final out)
"""
import math
from contextlib import ExitStack

import numpy as np

import concourse.tile as tile
from concourse import bacc, mybir
from concourse.masks import make_identity
from concourse.bass_utils import run_bass_kernel_spmd

S = 2048          # sequence length
D = 2048          # model dim
H = 16            # heads
DH = 64           # head dim (per component); 2*DH = 128 channels per head
NCORES = 8
HPC = H // NCORES          # heads per core = 2
CPC = HPC * 2 * DH         # channels per core = 256
LAMBDA_INIT = 0.8
EPS = 1e-5

DBLK = 512                 # s-block width
NB = S // DBLK             # 4 s-blocks
KT = D // 128              # 16 k-tiles (contraction for projections)
TT = S // 128              # 16 t-tiles (keys)
F32R = mybir.dt.float32r
F32 = mybir.dt.float32
I32 = mybir.dt.int32
BF16 = mybir.dt.bfloat16
Exp = mybir.ActivationFunctionType.Exp
Op = mybir.AluOpType


def _build(lam: float):
    nc = bacc.Bacc("TRN2", target_bir_lowering=False, debug=False,
                   num_devices=NCORES)

    xt_d = nc.dram_tensor("xt", [KT, NB, 128, DBLK], F32R, kind="ExternalInput").ap()
    # weights grouped by 4 k-tiles: [g, p, k_in_g * CPC] -> 16KB DMA rows
    wq_d = nc.dram_tensor("wq", [KT // 4, 128, 4 * CPC], F32R, kind="ExternalInput").ap()
    wk_d = nc.dram_tensor("wk", [KT // 4, 128, 4 * CPC], F32R, kind="ExternalInput").ap()
    wv_d = nc.dram_tensor("wv", [KT // 4, 128, 4 * CPC], F32R, kind="ExternalInput").ap()
    wo_d = nc.dram_tensor("wo", [HPC, 128, D], F32R, kind="ExternalInput").ap()
    out_d = nc.dram_tensor("out_p", [S, D], F32, kind="ExternalOutput").ap()

    with tile.TileContext(nc) as tc, ExitStack() as ctx:
        singles = ctx.enter_context(tc.tile_pool(name="singles", bufs=1))
        xt_pool = ctx.enter_context(tc.tile_pool(name="xt", bufs=6))
        exp_pool = ctx.enter_context(tc.tile_pool(name="exp", bufs=36))
        gn_pool = ctx.enter_context(tc.tile_pool(name="gn", bufs=5))
        ost_pool = ctx.enter_context(tc.tile_pool(name="ost", bufs=3))
        psA = ctx.enter_context(tc.tile_pool(name="psA", bufs=2, space="PSUM"))
        psB = ctx.enter_context(tc.tile_pool(name="psB", bufs=2, space="PSUM"))
        psC = ctx.enter_context(tc.tile_pool(name="psC", bufs=2, space="PSUM"))

        wq_sb = [singles.tile([128, 4, CPC], F32R, tag=f"wq{g}", name=f"wq{g}")
                 for g in range(KT // 4)]
        wk_sb = [singles.tile([128, 4, CPC], F32R, tag=f"wk{g}", name=f"wk{g}")
                 for g in range(KT // 4)]
        wv_sb = [singles.tile([128, 4, CPC], F32R, tag=f"wv{g}", name=f"wv{g}")
                 for g in range(KT // 4)]
        wo_sb = singles.tile([128, HPC, D], F32R, tag="wo")
        # k-group 0 on the fast HWDGE ring (ahead of the xt stream);
        # later groups + wo via SWDGE so they don't block xt tiles.
        nc.sync.dma_start(out=wk_sb[0], in_=wk_d[0])
        nc.sync.dma_start(out=wv_sb[0], in_=wv_d[0])
        nc.sync.dma_start(out=wq_sb[0], in_=wq_d[0])
        for g in range(1, KT // 4):
            nc.gpsimd.dma_start(out=wk_sb[g], in_=wk_d[g])
            nc.gpsimd.dma_start(out=wv_sb[g], in_=wv_d[g])
            nc.gpsimd.dma_start(out=wq_sb[g], in_=wq_d[g])
        for ct in range(HPC):
            nc.gpsimd.dma_start(out=wo_sb[:, ct, :], in_=wo_d[ct])

        # qT/kT per head: [128 rows = (q1 dims 0:64 | q2 dims 64:128), S]
        qT_sb = [singles.tile([128, S], F32R, tag=f"qT{h}", name=f"qT{h}")
                 for h in range(HPC)]
        kT_sb = [singles.tile([128, S], F32R, tag=f"kT{h}", name=f"kT{h}")
                 for h in range(HPC)]
        # v per t-tile: [128 t, 260]: h0 v 0:128, one 128, pad, h1 v 130:258, one 258
        v_sb = singles.tile([128, TT, 260], BF16, tag="v")
        nc.vector.memset(v_sb[:, :, 128:129], 1.0)
        nc.vector.memset(v_sb[:, :, 258:259], 1.0)

        ident = singles.tile([128, 128], F32, tag="ident")
        make_identity(nc, ident)
        magic = singles.tile([128, 2], I32, tag="magic")
        nc.vector.memset(magic, 0x5F3759DF)
        one_i = singles.tile([128, 1], I32, tag="one_i")
        nc.vector.memset(one_i, 1)

        # ---- Stage A1: projections (q,k,v in one xt stream) ----
        for b in range(NB):
            sblk = slice(b * DBLK, (b + 1) * DBLK)
            pq = psA.tile([128, 2, DBLK], F32, tag="A")
            pk = psA.tile([128, 2, DBLK], F32, tag="A")
            pv = [psB.tile([128, DBLK], F32, tag="B", name=f"pv{j}")
                  if j < 2 else
                  psC.tile([128, DBLK], F32, tag="C", name=f"pv{j}")
                  for j in range(4)]
            for k in range(KT):
                xt_t = xt_pool.tile([128, DBLK], F32R, tag="xt")
                nc.sync.dma_start(out=xt_t, in_=xt_d[k, b])
                st, sp = (k == 0), (k == KT - 1)
                g, ki = k // 4, k % 4
                for h in range(HPC):
                    nc.tensor.matmul(
                        pq[:, h, :],
                        wq_sb[g][:, ki, h * 128:(h + 1) * 128],
                        xt_t, start=st, stop=sp)
                    nc.tensor.matmul(
                        pk[:, h, :],
                        wk_sb[g][:, ki, h * 128:(h + 1) * 128],
                        xt_t, start=st, stop=sp)
                for j in range(4):
                    nc.tensor.matmul(pv[j][:, 0:CPC],
                                     xt_t[:, j * 128:(j + 1) * 128],
                                     wv_sb[g][:, ki, :], start=st, stop=sp)
            for h in range(HPC):
                nc.vector.tensor_copy(qT_sb[h][:, sblk], pq[:, h, :])
                nc.vector.tensor_copy(kT_sb[h][:, sblk], pk[:, h, :])
            for j in range(4):
                t_idx = b * 4 + j
                nc.vector.tensor_copy(v_sb[:, t_idx, 0:128], pv[j][:, 0:128])
                nc.vector.tensor_copy(v_sb[:, t_idx, 130:258], pv[j][:, 128:256])

        # ---- Attention per s-block, pipelined per head ----
        # Emission order per block: exp(h0), exp(h1), PV/GN(h0), PV/GN(h1),
        # final. The scheduler runs PV(h0) while ACT computes exp(h1), and
        # exp of the next (block, head) while PV(h1)/final run — keeping the
        # PE dense enough that HAM stays at full clock.
        def emit_scores(b, h):
            sblk = slice(b * DBLK, (b + 1) * DBLK)
            tiles = [[None] * 8, [None] * 8]
            for tp in range(8):
                s1 = psA.tile([128, 2, DBLK], F32, tag="A")
                s2 = psA.tile([128, 2, DBLK], F32, tag="A")
                for u in range(2):
                    t = 2 * tp + u
                    tsl = slice(t * 128, (t + 1) * 128)
                    nc.tensor.matmul(s1[:, u, :], kT_sb[h][0:64, tsl],
                                     qT_sb[h][0:64, sblk], start=True, stop=True)
                    nc.tensor.matmul(s2[:, u, :], kT_sb[h][64:128, tsl],
                                     qT_sb[h][64:128, sblk], start=True, stop=True)
                e1 = exp_pool.tile([128, 2, DBLK], BF16, tag="exp")
                e2 = exp_pool.tile([128, 2, DBLK], BF16, tag="exp")
                nc.scalar.activation(e1, s1, Exp)
                nc.scalar.activation(e2, s2, Exp)
                tiles[0][tp] = e1
                tiles[1][tp] = e2
            return tiles

        def emit_pv_gn(b, h, j, exp_tiles):
            """PV + GN for (block, head, s-tile j) -> [c,s] f32r tile."""
            jsl = slice(j * 128, (j + 1) * 128)
            vsl = slice(h * 130, h * 130 + 129)
            O1t = psB.tile([128, DBLK], F32, tag="B", name="O1t")
            O2t = psB.tile([128, DBLK], F32, tag="B", name="O2t")
            Oc = None
            for t in range(TT):
                st, sp = (t == 0), (t == TT - 1)
                nc.tensor.matmul(O1t[:, 0:129], exp_tiles[0][t // 2][:, t % 2, jsl],
                                 v_sb[:, t, vsl], start=st, stop=sp)
                nc.tensor.matmul(O2t[:, 0:129], exp_tiles[1][t // 2][:, t % 2, jsl],
                                 v_sb[:, t, vsl], start=st, stop=sp)
            dd = gn_pool.tile([128, 2], F32, tag="dd")
            nc.vector.tensor_copy(dd[:, 0:1], O1t[:, 128:129])
            nc.vector.tensor_copy(dd[:, 1:2], O2t[:, 128:129])
            rec2 = gn_pool.tile([128, 1], F32, tag="rec2")
            nc.vector.reciprocal(rec2, dd[:, 1:2])
            rneg = gn_pool.tile([128, 1], F32, tag="rneg")
            nc.vector.tensor_scalar(
                out=rneg, in0=rec2, scalar1=dd[:, 0:1], scalar2=-lam,
                op0=Op.mult, op1=Op.mult)
            z = gn_pool.tile([128, 128], F32, tag="z")
            nc.vector.tensor_copy(z, O1t[:, 0:128])
            nc.vector.scalar_tensor_tensor(
                out=z, in0=O2t[:, 0:128], scalar=rneg, in1=z,
                op0=Op.mult, op1=Op.add)
            stats = gn_pool.tile([128, 6], F32, tag="stats")
            nc.vector.bn_stats(out=stats, in_=z)
            mv = gn_pool.tile([128, 2], F32, tag="mv")
            nc.vector.bn_aggr(out=mv, in_=stats)
            # w = var + eps*d1^2 ; rstd = rsqrt(w) via bit-seed + 2 Newton iters
            ww = gn_pool.tile([128, 1], F32, tag="ww")
            nc.vector.tensor_scalar(
                out=ww, in0=dd[:, 0:1], scalar1=dd[:, 0:1], scalar2=EPS,
                op0=Op.mult, op1=Op.mult)
            nc.vector.tensor_tensor(out=ww, in0=mv[:, 1:2], in1=ww, op=Op.add)
            sh = gn_pool.tile([128, 1], I32, tag="sh")
            nc.vector.tensor_scalar(
                out=sh, in0=ww.bitcast(I32), scalar1=one_i,
                scalar2=None, op0=Op.arith_shift_right)
            yy = gn_pool.tile([128, 1], F32, tag="yy")
            nc.vector.tensor_tensor(
                out=yy.bitcast(I32), in0=magic[:, 0:1], in1=sh, op=Op.subtract)
            for _ in range(2):
                y2 = gn_pool.tile([128, 1], F32, tag="y2")
                nc.vector.tensor_tensor(out=y2, in0=yy, in1=yy, op=Op.mult)
                nc.vector.tensor_tensor(out=y2, in0=y2, in1=ww, op=Op.mult)
                nc.vector.tensor_scalar(
                    out=y2, in0=y2, scalar1=-0.5, scalar2=1.5,
                    op0=Op.mult, op1=Op.add)
                nyy = gn_pool.tile([128, 1], F32, tag="yy")
                nc.vector.tensor_tensor(out=nyy, in0=yy, in1=y2, op=Op.mult)
                yy = nyy
            xh = gn_pool.tile([128, 128], F32R, tag="xh")
            nc.vector.tensor_scalar(
                out=xh, in0=z, scalar1=mv[:, 0:1], scalar2=yy,
                op0=Op.subtract, op1=Op.mult)
            trp = psC.tile([128, DBLK], F32, tag="C", name="trp")
            nc.tensor.transpose(trp[:, 0:128], xh.bitcast(F32), ident)
            tr = gn_pool.tile([128, 128], F32R, tag="tr", bufs=10)
            nc.vector.tensor_copy(tr, trp[:, 0:128])
            return tr

        def emit_final(b, j, trs_j):
            srow = (b * 4 + j) * 128
            for n in range(4):
                po = psC.tile([128, DBLK], F32, tag="C", name="po")
                dsl = slice(n * DBLK, (n + 1) * DBLK)
                for ct in range(HPC):
                    nc.tensor.matmul(po, trs_j[ct], wo_sb[:, ct, dsl],
                                     start=(ct == 0), stop=(ct == HPC - 1))
                ostage = ost_pool.tile([128, DBLK], F32, tag="ost")
                nc.vector.tensor_copy(ostage, po)
                nc.sync.dma_start(out=out_d[srow:srow + 128, dsl], in_=ostage)

        exp_cur = [emit_scores(0, h) for h in range(HPC)]
        for b in range(NB):
            trs = [[None] * HPC for _ in range(4)]
            for j in range(4):
                trs[j][0] = emit_pv_gn(b, 0, j, exp_cur[0])
            if b + 1 < NB:
                exp_cur[0] = emit_scores(b + 1, 0)
            for j in range(4):
                trs[j][1] = emit_pv_gn(b, 1, j, exp_cur[1])
                emit_final(b, j, trs[j])
            if b + 1 < NB:
                exp_cur[1] = emit_scores(b + 1, 1)

    nc.compile()
    return nc


def prepare(x, Wq, Wk, Wv, Wo, lambda_q1, lambda_k1, lambda_q2, lambda_k2,
            gn_weight, gn_bias):
    """Host-side sharding/preprocessing. Returns (lam, in_maps, bias_vec)."""
    x = np.asarray(x, dtype=np.float32)
    Wq = np.asarray(Wq, dtype=np.float32)
    Wk = np.asarray(Wk, dtype=np.float32)
    Wv = np.asarray(Wv, dtype=np.float32)
    Wo = np.asarray(Wo, dtype=np.float32)
    gw = np.asarray(gn_weight, dtype=np.float32)
    gb = np.asarray(gn_bias, dtype=np.float32)

    lam = float(np.exp(np.sum(np.asarray(lambda_q1, np.float64)
                              * np.asarray(lambda_k1, np.float64)))
                - np.exp(np.sum(np.asarray(lambda_q2, np.float64)
                                * np.asarray(lambda_k2, np.float64)))
                + LAMBDA_INIT)

    xT = np.ascontiguousarray(
        x.reshape(S, D).T.reshape(KT, 128, NB, DBLK).transpose(0, 2, 1, 3))
    scale = 1.0 / math.sqrt(DH)

    in_maps = []
    for c in range(NCORES):
        sl = slice(c * CPC, (c + 1) * CPC)
        def _grp(w):
            return np.ascontiguousarray(
                w.reshape(KT // 4, 4, 128, CPC).transpose(0, 2, 1, 3)
                .reshape(KT // 4, 128, 4 * CPC))
        wq_c = _grp(Wq[:, sl] * scale)
        wk_c = _grp(Wk[:, sl])
        wv_c = _grp(Wv[:, sl])
        wo_c = np.ascontiguousarray(
            Wo[sl, :] * ((1.0 - LAMBDA_INIT) * gw[sl])[:, None]
        ).reshape(HPC, 128, D)
        in_maps.append({"xt": xT, "wq": wq_c, "wk": wk_c, "wv": wv_c,
                        "wo": wo_c})

    bias_vec = ((1.0 - LAMBDA_INIT) * gb.astype(np.float64)) @ Wo.astype(np.float64)
    return lam, in_maps, bias_vec


def kernel(x, Wq, Wk, Wv, Wo, lambda_q1, lambda_k1, lambda_q2, lambda_k2,
           gn_weight, gn_bias):
    lam, in_maps, bias_vec = prepare(
        x, Wq, Wk, Wv, Wo, lambda_q1, lambda_k1, lambda_q2, lambda_k2,
        gn_weight, gn_bias)
    nc = _build(lam)
    res = run_bass_kernel_spmd(nc, in_maps, list(range(NCORES)))
    acc = np.zeros((S, D), dtype=np.float64)
    for c in range(NCORES):
        acc += res.results[c]["out_p"]
    acc += bias_vec[None, :]
    return acc.astype(np.float32).reshape(1, S, D)
